# revision 1
# baseline (speedup 1.0000x reference)
"""Trainium2 Bass kernel for nn_MultiHeadAttention (B=4, S=2048, D=1024, H=16, HD=64).

Sharding: 8 cores = 4 batches (data parallel) x 2 head-groups of 8 heads
(tensor parallel). Each core computes its batch's QKV projections for its 8
heads, full softmax attention, and the partial output projection for its head
group. The host sums the two head-group partials per batch (the hinted
all-reduce, done at gather time) and adds the output bias.

Per-core kernel layout (all matmuls in float32r at N=512 -> full PE rate):
  - Host pre-transposes activations to X^T [D, S] so projections stream
    contiguously with the contraction dim (features) on partitions.
  - Q/K are produced transposed, Q^T/K^T [dh, tok], one [128, 2048] SBUF tile
    per head pair (head A partitions 0:64, head B 64:128). QKV biases are
    applied for free via the ACT psum->sbuf copy (per-partition bias).
  - Scores are computed transposed S^T[k, q] per head; the two heads of a pair
    run concurrently via PE row packing (tile_position (0,0)/(64,0), K=64).
  - exp runs on ACT straight out of 2-bank PSUM tiles [128, 1024] into an
    E^T SBUF tile [128, 16*512] per head (softmax scale folded into exp).
  - PV: lhsT = V' [128, 65] where column 64 is ones, so the PSUM accumulator
    row 64 collects the softmax denominator Z in the same pass.
  - Normalize: DVE reciprocal of Z + GPSIMD partition_broadcast + DVE multiply,
    writing O^T tiles [dh, tok] which feed the output projection as lhsT.

SBUF pressure is managed with phase-scoped tile pools on the queue allocator:
V'-staging pools die before the Q/K projection pools, which die before the
attention pools (E^T is 64KB/partition), which die before the out-proj pools.
V' and O^T round-trip through DRAM scratch to keep residency low.
"""

import numpy as np
from contextlib import ExitStack

B, S, D = 4, 2048, 1024
H, HD = 16, 64
NCORES = 8
HPC = H // 2            # heads per core = 8
PAIRS = HPC // 2        # head pairs per core = 4
DH = HPC * HD           # per-core head dims = 512
P = 128
TOK_T = S // P          # 16 token tiles of 128
QC = S // 512           # 4 query chunks of 512
KC = S // P             # 16 key chunks of 128
KCG = KC // 2           # 8 exp groups of 2 key chunks
FC = D // P             # 8 feature chunks of 128

_CACHE = {}


def _tf32(x):
    """Round fp32 -> fp32r (tfloat32: 11-bit mantissa, low 12 bits zero, RNE).
    Matches walrus fp32_to_fp32r, so DRAM inputs can be declared float32r."""
    b = np.ascontiguousarray(x, np.float32).view(np.uint32)
    bias = np.uint32(0x7FF) + ((b >> np.uint32(12)) & np.uint32(1))
    b = (b + bias) & np.uint32(0xFFFFF000)
    return b.view(np.float32)


def _build(reps=1):
    import concourse.bacc as bacc
    import concourse.mybir as mybir
    import concourse.tile as tile

    dt = mybir.dt
    f32 = dt.float32
    f32r = dt.float32r
    AF = mybir.ActivationFunctionType

    nc = bacc.Bacc("TRN2", target_bir_lowering=False, debug=False)

    xqT = nc.dram_tensor("xqT", [D, S], f32r, kind="ExternalInput")
    xkT = nc.dram_tensor("xkT", [D, S], f32r, kind="ExternalInput")
    xvT = nc.dram_tensor("xvT", [D, S], f32r, kind="ExternalInput")
    wq = nc.dram_tensor("wq", [D, DH], f32r, kind="ExternalInput")
    wk = nc.dram_tensor("wk", [D, DH], f32r, kind="ExternalInput")
    wv = nc.dram_tensor("wv", [D, DH], f32r, kind="ExternalInput")
    wo = nc.dram_tensor("wo", [DH, D], f32r, kind="ExternalInput")
    biases = nc.dram_tensor("biases", [P, 3 * PAIRS], f32, kind="ExternalInput")
    out = nc.dram_tensor("out", [S, D], f32, kind="ExternalOutput")

    QCC = 4          # query chunks of 512
    QW = S // QCC    # 512
    KPG = 2          # key tiles per exp group
    NG = KC // KPG   # 8 exp groups per (head, qc)

    def mmr(psum, lhsT, rhs, **kw):
        nc.tensor.matmul(psum, lhsT, rhs, **kw)

    with tile.TileContext(nc, pool_alloc_mode="queue") as tc, ExitStack() as ctx:
        # ---- persistent pools ----
        qt_pool = ctx.enter_context(tc.tile_pool(name="qt", bufs=PAIRS))
        kt_pool = ctx.enter_context(tc.tile_pool(name="kt", bufs=PAIRS))
        vpr_pool = ctx.enter_context(tc.tile_pool(name="vpr", bufs=TOK_T))
        ot_pool = ctx.enter_context(tc.tile_pool(name="ot", bufs=5))
        zr_pool = ctx.enter_context(tc.tile_pool(name="zr", bufs=2))
        zb_pool = ctx.enter_context(tc.tile_pool(name="zb", bufs=2))
        bias_pool = ctx.enter_context(tc.tile_pool(name="bias", bufs=1))
        dram_pool = ctx.enter_context(tc.tile_pool(name="dram", bufs=1, space="DRAM"))
        psp = ctx.enter_context(tc.tile_pool(name="psp", bufs=2, space="PSUM"))
        pssc = ctx.enter_context(tc.tile_pool(name="pssc", bufs=2, space="PSUM"))
        pso = ctx.enter_context(tc.tile_pool(name="pso", bufs=2, space="PSUM"))

        SCALE = 1.0 / float(np.sqrt(HD))
        for rep in range(reps):
          qt_t = [qt_pool.tile([P, S], f32r, name=f"qt_{rep}_{p}", tag="qt")
                  for p in range(PAIRS)]
          kt_t = [kt_pool.tile([P, S], f32r, name=f"kt_{rep}_{p}", tag="kt")
                  for p in range(PAIRS)]
          bias_t = bias_pool.tile([P, 3 * PAIRS], f32, name=f"bias_{rep}", tag="bias")
          nc.sync.dma_start(bias_t[:], biases[:])
          bq_t = {p: bias_t[:, p:p + 1] for p in range(PAIRS)}
          bk_t = {p: bias_t[:, PAIRS + p:PAIRS + p + 1] for p in range(PAIRS)}
          bv_t = {p: bias_t[:, 2 * PAIRS + p:2 * PAIRS + p + 1] for p in range(PAIRS)}

          # ---- attention helpers ----
          def alloc_et(p, qc):
              et = {}
              for hh in range(2):
                  for quarter in range(4):
                      et[(hh, quarter)] = et_pool.tile(
                          [P, 4 * QW], f32r,
                          name=f"et_{rep}_{p}_{qc}_{hh}_{quarter}", tag="et")
              return et

          def scores_exp_group(p, qc, g, et):
              half, goff = g // 2, (g % 2) * KPG * QW  # quarter idx, offset
              psA = pssc.tile([P, KPG * QW], f32,
                              name=f"scA_{rep}_{p}_{qc}_{g}", tag="pssc")
              psB = pssc.tile([P, KPG * QW], f32,
                              name=f"scB_{rep}_{p}_{qc}_{g}", tag="pssc")
              for j in range(KPG):
                  kc = g * KPG + j
                  nc.tensor.matmul(
                      psA[:, j * QW:(j + 1) * QW],
                      kt_t[p][0:64, kc * P:(kc + 1) * P],
                      qt_t[p][0:64, qc * QW:(qc + 1) * QW],
                      start=True, stop=True, tile_position=(0, 0))
                  nc.tensor.matmul(
                      psB[:, j * QW:(j + 1) * QW],
                      kt_t[p][64:128, kc * P:(kc + 1) * P],
                      qt_t[p][64:128, qc * QW:(qc + 1) * QW],
                      start=True, stop=True, tile_position=(64, 0))
              nc.scalar.activation(et[(0, half)][:, goff:goff + KPG * QW],
                                   psA[:], AF.Exp, scale=SCALE)
              nc.scalar.activation(et[(1, half)][:, goff:goff + KPG * QW],
                                   psB[:], AF.Exp, scale=SCALE)

          def scores_exp(p, qc):
              et = alloc_et(p, qc)
              for g in range(NG):
                  scores_exp_group(p, qc, g, et)
              return et

          def pv_chunk(p, qc, et, po_pair, kcs):
              poA, poB = po_pair
              for kc in kcs:
                  half, koff = kc // 4, (kc % 4) * QW
                  cA = (2 * p) * (HD + 1)
                  cB = (2 * p + 1) * (HD + 1)
                  mmr(poA[:], vpr_t[kc][:, cA:cA + HD + 1],
                      et[(0, half)][:, koff:koff + QW],
                      start=(kc == 0), stop=(kc == KC - 1))
                  mmr(poB[:], vpr_t[kc][:, cB:cB + HD + 1],
                      et[(1, half)][:, koff:koff + QW],
                      start=(kc == 0), stop=(kc == KC - 1))

          def normalize(p, qc, po_pair):
              poA, poB = po_pair
              ot_t = ot_pool.tile([P, QW], f32r, name=f"ot_{rep}_{p}_{qc}", tag="ot")
              for hh, po in ((0, poA), (1, poB)):
                  zr = zr_pool.tile([1, QW], f32,
                                    name=f"zr_{rep}_{p}_{qc}_{hh}", tag="zr")
                  nc.vector.reciprocal(zr[:], po[64:65, :])
                  zb = zb_pool.tile([64, QW], f32,
                                    name=f"zb_{rep}_{p}_{qc}_{hh}", tag="zb")
                  nc.gpsimd.partition_broadcast(zb[:], zr[:])
                  nc.vector.tensor_mul(ot_t[hh * 64:(hh + 1) * 64, :],
                                       po[0:64, :], zb[:])
              nc.vector.tensor_scalar_add(ot_t[:], ot_t[:], bv_t[p][:])
              return ot_t

          def attention(p, qc):
              et = scores_exp(p, qc)
              poA = pso.tile([65, QW], f32, name=f"poA_{rep}_{p}_{qc}", tag="pso")
              poB = pso.tile([65, QW], f32, name=f"poB_{rep}_{p}_{qc}", tag="pso")
              pv_chunk(p, qc, et, (poA, poB), range(KC))
              return normalize(p, qc, (poA, poB))

          vpr_t = []
          # ---- V projection into resident V' tiles (runs first) ----
          with tc.tile_pool(name="wvp", bufs=FC) as wv_pool, \
               tc.tile_pool(name="xsv", bufs=10) as xsv_pool:
              wv_t = []
              for f in range(FC):
                  t = wv_pool.tile([P, 512], f32r, name=f"wv_{rep}_{f}", tag="wv")
                  nc.sync.dma_start(t[:], wv[f * P:(f + 1) * P, :])
                  wv_t.append(t)
              for tcg in range(TOK_T // 4):
                  xv_t = []
                  for f in range(FC):
                      t = xsv_pool.tile([P, 512], f32r,
                                        name=f"xv_{rep}_{tcg}_{f}", tag="xsv")
                      nc.sync.dma_start(t[:], xvT[f * P:(f + 1) * P,
                                                  tcg * 512:(tcg + 1) * 512])
                      xv_t.append(t)
                  for tl in range(4):
                      tci = tcg * 4 + tl
                      ps = psp.tile([P, 512], f32, name=f"psv_{rep}_{tci}", tag="psp")
                      for f in range(FC):
                          mmr(ps[:], xv_t[f][:, tl * P:(tl + 1) * P], wv_t[f][:],
                              start=(f == 0), stop=(f == FC - 1))
                      vt = vpr_pool.tile([P, HPC * (HD + 1)], f32r,
                                         name=f"vpr_{rep}_{tci}", tag="vpr")
                      v3 = vt.rearrange("p (h c) -> p h c", c=HD + 1)
                      nc.gpsimd.memset(v3[:, :, HD:HD + 1].bitcast(f32), 1.0)
                      nc.vector.tensor_copy(v3[:, :, 0:HD],
                                            ps.rearrange("p (h c) -> p h c", c=HD))
                      vpr_t.append(vt)

          # ---- Q^T / K^T projections (stream X^T once; all pairs) ----
          with tc.tile_pool(name="xs", bufs=10) as xs_pool, \
               tc.tile_pool(name="wqk", bufs=2 * FC) as wqk_pool:
              wq_t, wk_t = {}, {}
              for tc4 in range(QC):
                  for (xT, wT, w_t, dst, b_t, nm) in (
                          (xqT, wq, wq_t, qt_t, bq_t, "q"),
                          (xkT, wk, wk_t, kt_t, bk_t, "k")):
                      x_t = []
                      for f in range(FC):
                          if tc4 == 0:
                              t = wqk_pool.tile([P, 512], f32r,
                                                name=f"w{nm}_{rep}_{f}", tag="wqk")
                              nc.sync.dma_start(t[:], wT[f * P:(f + 1) * P, :])
                              w_t[f] = t
                          t = xs_pool.tile([P, 512], f32r,
                                           name=f"x{nm}_{rep}_{tc4}_{f}", tag="xs")
                          nc.sync.dma_start(t[:], xT[f * P:(f + 1) * P,
                                                     tc4 * 512:(tc4 + 1) * 512])
                          x_t.append(t)
                      for p in range(PAIRS):
                          ps = psp.tile([P, 512], f32,
                                        name=f"ps{nm}_{rep}_{tc4}_{p}", tag="psp")
                          for f in range(FC):
                              mmr(ps[:], w_t[f][:, p * P:(p + 1) * P], x_t[f][:],
                                  start=(f == 0), stop=(f == FC - 1))
                          nc.vector.tensor_scalar_add(
                              dst[p][:, tc4 * 512:(tc4 + 1) * 512], ps[:], b_t[p][:])


          # ---- qc-outer attention sweep; out-proj absorbed per qc unit ----
          with tc.tile_pool(name="et", bufs=8) as et_pool, \
               tc.tile_pool(name="wop", bufs=2 * PAIRS) as wo_pool, \
               tc.tile_pool(name="os", bufs=3) as os_pool:
              wo_t = {}
              for p in range(PAIRS):
                  for dc in range(2):
                      t = wo_pool.tile([P, 512], f32r,
                                       name=f"wo_{rep}_{p}_{dc}", tag="wo")
                      nc.sync.dma_start(t[:], wo[p * P:(p + 1) * P,
                                                 dc * 512:(dc + 1) * 512])
                      wo_t[(p, dc)] = t

              def outproj(qc, ots):
                  for tl in range(QW // P):
                      tci = qc * (QW // P) + tl
                      for dc in range(2):
                          ps = psp.tile([P, 512], f32,
                                        name=f"pout_{rep}_{tci}_{dc}", tag="psp")
                          for pp in range(PAIRS):
                              mmr(ps[:], ots[pp][:, tl * P:(tl + 1) * P],
                                  wo_t[(pp, dc)][:],
                                  start=(pp == 0), stop=(pp == PAIRS - 1))
                          ost = os_pool.tile([P, 512], f32,
                                             name=f"os_{rep}_{tci}_{dc}", tag="os")
                          nc.vector.tensor_copy(ost[:], ps[:])
                          nc.sync.dma_start(out[tci * P:(tci + 1) * P,
                                                dc * 512:(dc + 1) * 512], ost[:])

              for qc in range(QCC):
                  ots = []
                  for p in range(PAIRS):
                      et = scores_exp(p, qc)
                      poA = pso.tile([65, QW], f32,
                                     name=f"poA_{rep}_{p}_{qc}", tag="pso")
                      poB = pso.tile([65, QW], f32,
                                     name=f"poB_{rep}_{p}_{qc}", tag="pso")
                      pv_chunk(p, qc, et, (poA, poB), range(KC))
                      ots.append(normalize(p, qc, (poA, poB)))
                  outproj(qc, ots)
    nc.compile()
    return nc


def _get_nc(reps=1):
    if reps not in _CACHE:
        _CACHE[reps] = _build(reps)
    return _CACHE[reps]


def _in_maps(inputs):
    f = np.float32
    maps = []
    for c in range(NCORES):
        b, g = c // 2, c % 2
        hs = slice(g * HPC, (g + 1) * HPC)
        maps.append({
            "xqT": _tf32(np.asarray(inputs["inputs_q"][b], f).T),
            "xkT": _tf32(np.asarray(inputs["inputs_k"][b], f).T),
            "xvT": _tf32(np.asarray(inputs["inputs_v"][b], f).T),
            "wq": _tf32(np.asarray(inputs["Wq"], f)[:, hs, :].reshape(D, DH)),
            "wk": _tf32(np.asarray(inputs["Wk"], f)[:, hs, :].reshape(D, DH)),
            "wv": _tf32(np.asarray(inputs["Wv"], f)[:, hs, :].reshape(D, DH)),
            "wo": _tf32(np.asarray(inputs["Wo"], f)[hs].reshape(DH, D)),
            "biases": np.stack(
                [np.asarray(inputs[nm], f)[hs].reshape(DH)[p * P:(p + 1) * P]
                 for nm in ("bq", "bk", "bv") for p in range(PAIRS)], axis=1).copy(),
        })
    return maps


def run_sharded(inputs, **kw):
    """Compile/run on all 8 cores; returns (full_output, BassKernelResults)."""
    from concourse.bass_utils import run_bass_kernel_spmd
    nc = _get_nc()
    res = run_bass_kernel_spmd(nc, _in_maps(inputs), core_ids=list(range(NCORES)), **kw)
    bo = np.asarray(inputs["bo"], np.float32)
    full = np.empty((B, S, D), np.float32)
    for b in range(B):
        full[b] = res.results[2 * b]["out"] + res.results[2 * b + 1]["out"] + bo
    return full, res


def kernel(**inputs) -> np.ndarray:
    full, _ = run_sharded(inputs)
    return full



# revision 3
# speedup vs baseline: 1.2601x; 1.2601x over previous
"""Trainium2 Bass kernel for nn_MultiHeadAttention (B=4, S=2048, D=1024, H=16, HD=64).

Sharding: 8 cores = 4 batches (data parallel) x 2 head-groups of 8 heads
(tensor parallel). Each core computes its batch's QKV projections for its 8
heads, full softmax attention, and the partial output projection for its head
group. The host sums the two head-group partials per batch (the hinted
all-reduce, done at gather time) and adds the output bias.

Per-core kernel layout (all matmuls in float32r at N=512 -> full PE rate):
  - Host pre-transposes activations to X^T [D, S] so projections stream
    contiguously with the contraction dim (features) on partitions.
  - Q/K are produced transposed, Q^T/K^T [dh, tok], one [128, 2048] SBUF tile
    per head pair (head A partitions 0:64, head B 64:128). QKV biases are
    applied for free via the ACT psum->sbuf copy (per-partition bias).
  - Scores are computed transposed S^T[k, q] per head; the two heads of a pair
    run concurrently via PE row packing (tile_position (0,0)/(64,0), K=64).
  - exp runs on ACT straight out of 2-bank PSUM tiles [128, 1024] into an
    E^T SBUF tile [128, 16*512] per head (softmax scale folded into exp).
  - PV: lhsT = V' [128, 65] where column 64 is ones, so the PSUM accumulator
    row 64 collects the softmax denominator Z in the same pass.
  - Normalize: DVE reciprocal of Z + GPSIMD partition_broadcast + DVE multiply,
    writing O^T tiles [dh, tok] which feed the output projection as lhsT.

SBUF pressure is managed with phase-scoped tile pools on the queue allocator:
V'-staging pools die before the Q/K projection pools, which die before the
attention pools (E^T is 64KB/partition), which die before the out-proj pools.
V' and O^T round-trip through DRAM scratch to keep residency low.
"""

import numpy as np
from contextlib import ExitStack

B, S, D = 4, 2048, 1024
H, HD = 16, 64
NCORES = 8
HPC = H // 2            # heads per core = 8
PAIRS = HPC // 2        # head pairs per core = 4
DH = HPC * HD           # per-core head dims = 512
P = 128
TOK_T = S // P          # 16 token tiles of 128
QC = S // 512           # 4 query chunks of 512
KC = S // P             # 16 key chunks of 128
KCG = KC // 2           # 8 exp groups of 2 key chunks
FC = D // P             # 8 feature chunks of 128

_CACHE = {}


def _bf16(x):
    """Round fp32 -> bf16 (RNE) on host so DRAM inputs are bfloat16.
    Halves DMA bytes and, more importantly, halves PE datapath toggling —
    the fp32r baseline spent most of the attention phase HAM-throttled to
    half clock; bf16 operands keep the PE at full rate."""
    import ml_dtypes
    return np.ascontiguousarray(x, np.float32).astype(ml_dtypes.bfloat16)


def _build(reps=1):
    import concourse.bacc as bacc
    import concourse.mybir as mybir
    import concourse.tile as tile

    dt = mybir.dt
    f32 = dt.float32
    bf16 = dt.bfloat16
    AF = mybir.ActivationFunctionType

    nc = bacc.Bacc("TRN2", target_bir_lowering=False, debug=False)

    xqT = nc.dram_tensor("xqT", [D, S], bf16, kind="ExternalInput")
    xkT = nc.dram_tensor("xkT", [D, S], bf16, kind="ExternalInput")
    xvT = nc.dram_tensor("xvT", [D, S], bf16, kind="ExternalInput")
    wq = nc.dram_tensor("wq", [D, DH], bf16, kind="ExternalInput")
    wk = nc.dram_tensor("wk", [D, DH], bf16, kind="ExternalInput")
    wv = nc.dram_tensor("wv", [D, DH], bf16, kind="ExternalInput")
    wo = nc.dram_tensor("wo", [DH, D], bf16, kind="ExternalInput")
    biases = nc.dram_tensor("biases", [P, 3 * PAIRS], f32, kind="ExternalInput")
    out = nc.dram_tensor("out", [S, D], f32, kind="ExternalOutput")

    QCC = 4          # query chunks of 512
    QW = S // QCC    # 512
    KPG = 2          # key tiles per exp group
    NG = KC // KPG   # 8 exp groups per (head, qc)

    def mmr(psum, lhsT, rhs, **kw):
        nc.tensor.matmul(psum, lhsT, rhs, **kw)

    with tile.TileContext(nc, pool_alloc_mode="queue") as tc, ExitStack() as ctx:
        # ---- persistent pools ----
        qt_pool = ctx.enter_context(tc.tile_pool(name="qt", bufs=PAIRS))
        kt_pool = ctx.enter_context(tc.tile_pool(name="kt", bufs=PAIRS))
        vpr_pool = ctx.enter_context(tc.tile_pool(name="vpr", bufs=TOK_T))
        ot_pool = ctx.enter_context(tc.tile_pool(name="ot", bufs=5))
        zr_pool = ctx.enter_context(tc.tile_pool(name="zr", bufs=2))
        zb_pool = ctx.enter_context(tc.tile_pool(name="zb", bufs=2))
        bias_pool = ctx.enter_context(tc.tile_pool(name="bias", bufs=1))
        dram_pool = ctx.enter_context(tc.tile_pool(name="dram", bufs=1, space="DRAM"))
        psp = ctx.enter_context(tc.tile_pool(name="psp", bufs=2, space="PSUM"))
        pssc = ctx.enter_context(tc.tile_pool(name="pssc", bufs=2, space="PSUM"))
        pso = ctx.enter_context(tc.tile_pool(name="pso", bufs=2, space="PSUM"))

        SCALE = 1.0 / float(np.sqrt(HD))
        for rep in range(reps):
          qt_t = [qt_pool.tile([P, S], bf16, name=f"qt_{rep}_{p}", tag="qt")
                  for p in range(PAIRS)]
          kt_t = [kt_pool.tile([P, S], bf16, name=f"kt_{rep}_{p}", tag="kt")
                  for p in range(PAIRS)]
          bias_t = bias_pool.tile([P, 3 * PAIRS], f32, name=f"bias_{rep}", tag="bias")
          nc.sync.dma_start(bias_t[:], biases[:])
          bq_t = {p: bias_t[:, p:p + 1] for p in range(PAIRS)}
          bk_t = {p: bias_t[:, PAIRS + p:PAIRS + p + 1] for p in range(PAIRS)}
          bv_t = {p: bias_t[:, 2 * PAIRS + p:2 * PAIRS + p + 1] for p in range(PAIRS)}

          # ---- attention helpers ----
          def alloc_et(p, qc):
              et = {}
              for hh in range(2):
                  for quarter in range(4):
                      et[(hh, quarter)] = et_pool.tile(
                          [P, 4 * QW], bf16,
                          name=f"et_{rep}_{p}_{qc}_{hh}_{quarter}", tag="et")
              return et

          def scores_exp_group(p, qc, g, et):
              half, goff = g // 2, (g % 2) * KPG * QW  # quarter idx, offset
              psA = pssc.tile([P, KPG * QW], f32,
                              name=f"scA_{rep}_{p}_{qc}_{g}", tag="pssc")
              psB = pssc.tile([P, KPG * QW], f32,
                              name=f"scB_{rep}_{p}_{qc}_{g}", tag="pssc")
              for j in range(KPG):
                  kc = g * KPG + j
                  nc.tensor.matmul(
                      psA[:, j * QW:(j + 1) * QW],
                      kt_t[p][0:64, kc * P:(kc + 1) * P],
                      qt_t[p][0:64, qc * QW:(qc + 1) * QW],
                      start=True, stop=True, tile_position=(0, 0))
                  nc.tensor.matmul(
                      psB[:, j * QW:(j + 1) * QW],
                      kt_t[p][64:128, kc * P:(kc + 1) * P],
                      qt_t[p][64:128, qc * QW:(qc + 1) * QW],
                      start=True, stop=True, tile_position=(64, 0))
              nc.scalar.activation(et[(0, half)][:, goff:goff + KPG * QW],
                                   psA[:], AF.Exp, scale=SCALE)
              nc.scalar.activation(et[(1, half)][:, goff:goff + KPG * QW],
                                   psB[:], AF.Exp, scale=SCALE)

          def scores_exp(p, qc):
              et = alloc_et(p, qc)
              for g in range(NG):
                  scores_exp_group(p, qc, g, et)
              return et

          def pv_chunk(p, qc, et, po_pair, kcs):
              poA, poB = po_pair
              for kc in kcs:
                  half, koff = kc // 4, (kc % 4) * QW
                  cA = (2 * p) * (HD + 1)
                  cB = (2 * p + 1) * (HD + 1)
                  mmr(poA[:], vpr_t[kc][:, cA:cA + HD + 1],
                      et[(0, half)][:, koff:koff + QW],
                      start=(kc == 0), stop=(kc == KC - 1))
                  mmr(poB[:], vpr_t[kc][:, cB:cB + HD + 1],
                      et[(1, half)][:, koff:koff + QW],
                      start=(kc == 0), stop=(kc == KC - 1))

          def normalize(p, qc, po_pair):
              poA, poB = po_pair
              ot_t = ot_pool.tile([P, QW], bf16, name=f"ot_{rep}_{p}_{qc}", tag="ot")
              for hh, po in ((0, poA), (1, poB)):
                  zr = zr_pool.tile([1, QW], f32,
                                    name=f"zr_{rep}_{p}_{qc}_{hh}", tag="zr")
                  nc.vector.reciprocal(zr[:], po[64:65, :])
                  zb = zb_pool.tile([64, QW], f32,
                                    name=f"zb_{rep}_{p}_{qc}_{hh}", tag="zb")
                  nc.gpsimd.partition_broadcast(zb[:], zr[:])
                  nc.vector.tensor_mul(ot_t[hh * 64:(hh + 1) * 64, :],
                                       po[0:64, :], zb[:])
              nc.vector.tensor_scalar_add(ot_t[:], ot_t[:], bv_t[p][:])
              return ot_t

          def attention(p, qc):
              et = scores_exp(p, qc)
              poA = pso.tile([65, QW], f32, name=f"poA_{rep}_{p}_{qc}", tag="pso")
              poB = pso.tile([65, QW], f32, name=f"poB_{rep}_{p}_{qc}", tag="pso")
              pv_chunk(p, qc, et, (poA, poB), range(KC))
              return normalize(p, qc, (poA, poB))

          vpr_t = []
          # ---- V projection into resident V' tiles (runs first) ----
          with tc.tile_pool(name="wvp", bufs=FC) as wv_pool, \
               tc.tile_pool(name="xsv", bufs=10) as xsv_pool:
              wv_t = []
              for f in range(FC):
                  t = wv_pool.tile([P, 512], bf16, name=f"wv_{rep}_{f}", tag="wv")
                  nc.sync.dma_start(t[:], wv[f * P:(f + 1) * P, :])
                  wv_t.append(t)
              for tcg in range(TOK_T // 4):
                  xv_t = []
                  for f in range(FC):
                      t = xsv_pool.tile([P, 512], bf16,
                                        name=f"xv_{rep}_{tcg}_{f}", tag="xsv")
                      nc.sync.dma_start(t[:], xvT[f * P:(f + 1) * P,
                                                  tcg * 512:(tcg + 1) * 512])
                      xv_t.append(t)
                  for tl in range(4):
                      tci = tcg * 4 + tl
                      ps = psp.tile([P, 512], f32, name=f"psv_{rep}_{tci}", tag="psp")
                      for f in range(FC):
                          mmr(ps[:], xv_t[f][:, tl * P:(tl + 1) * P], wv_t[f][:],
                              start=(f == 0), stop=(f == FC - 1))
                      vt = vpr_pool.tile([P, HPC * (HD + 1)], bf16,
                                         name=f"vpr_{rep}_{tci}", tag="vpr")
                      v3 = vt.rearrange("p (h c) -> p h c", c=HD + 1)
                      nc.gpsimd.memset(v3[:, :, HD:HD + 1], 1.0)
                      nc.vector.tensor_copy(v3[:, :, 0:HD],
                                            ps.rearrange("p (h c) -> p h c", c=HD))
                      vpr_t.append(vt)

          # ---- Q^T / K^T projections (stream X^T once; all pairs) ----
          with tc.tile_pool(name="xs", bufs=10) as xs_pool, \
               tc.tile_pool(name="wqk", bufs=2 * FC) as wqk_pool:
              wq_t, wk_t = {}, {}
              for tc4 in range(QC):
                  for (xT, wT, w_t, dst, b_t, nm) in (
                          (xqT, wq, wq_t, qt_t, bq_t, "q"),
                          (xkT, wk, wk_t, kt_t, bk_t, "k")):
                      x_t = []
                      for f in range(FC):
                          if tc4 == 0:
                              t = wqk_pool.tile([P, 512], bf16,
                                                name=f"w{nm}_{rep}_{f}", tag="wqk")
                              nc.sync.dma_start(t[:], wT[f * P:(f + 1) * P, :])
                              w_t[f] = t
                          t = xs_pool.tile([P, 512], bf16,
                                           name=f"x{nm}_{rep}_{tc4}_{f}", tag="xs")
                          nc.sync.dma_start(t[:], xT[f * P:(f + 1) * P,
                                                     tc4 * 512:(tc4 + 1) * 512])
                          x_t.append(t)
                      for p in range(PAIRS):
                          ps = psp.tile([P, 512], f32,
                                        name=f"ps{nm}_{rep}_{tc4}_{p}", tag="psp")
                          for f in range(FC):
                              mmr(ps[:], w_t[f][:, p * P:(p + 1) * P], x_t[f][:],
                                  start=(f == 0), stop=(f == FC - 1))
                          nc.vector.tensor_scalar_add(
                              dst[p][:, tc4 * 512:(tc4 + 1) * 512], ps[:], b_t[p][:])


          # ---- qc-outer attention sweep; out-proj absorbed per qc unit ----
          with tc.tile_pool(name="et", bufs=8) as et_pool, \
               tc.tile_pool(name="wop", bufs=2 * PAIRS) as wo_pool, \
               tc.tile_pool(name="os", bufs=3) as os_pool:
              wo_t = {}
              for p in range(PAIRS):
                  for dc in range(2):
                      t = wo_pool.tile([P, 512], bf16,
                                       name=f"wo_{rep}_{p}_{dc}", tag="wo")
                      nc.sync.dma_start(t[:], wo[p * P:(p + 1) * P,
                                                 dc * 512:(dc + 1) * 512])
                      wo_t[(p, dc)] = t

              def outproj(qc, ots):
                  for tl in range(QW // P):
                      tci = qc * (QW // P) + tl
                      for dc in range(2):
                          ps = psp.tile([P, 512], f32,
                                        name=f"pout_{rep}_{tci}_{dc}", tag="psp")
                          for pp in range(PAIRS):
                              mmr(ps[:], ots[pp][:, tl * P:(tl + 1) * P],
                                  wo_t[(pp, dc)][:],
                                  start=(pp == 0), stop=(pp == PAIRS - 1))
                          ost = os_pool.tile([P, 512], f32,
                                             name=f"os_{rep}_{tci}_{dc}", tag="os")
                          nc.vector.tensor_copy(ost[:], ps[:])
                          nc.sync.dma_start(out[tci * P:(tci + 1) * P,
                                                dc * 512:(dc + 1) * 512], ost[:])

              for qc in range(QCC):
                  ots = []
                  for p in range(PAIRS):
                      et = scores_exp(p, qc)
                      poA = pso.tile([65, QW], f32,
                                     name=f"poA_{rep}_{p}_{qc}", tag="pso")
                      poB = pso.tile([65, QW], f32,
                                     name=f"poB_{rep}_{p}_{qc}", tag="pso")
                      pv_chunk(p, qc, et, (poA, poB), range(KC))
                      ots.append(normalize(p, qc, (poA, poB)))
                  outproj(qc, ots)
    nc.compile()
    return nc


def _get_nc(reps=1):
    if reps not in _CACHE:
        _CACHE[reps] = _build(reps)
    return _CACHE[reps]


def _in_maps(inputs):
    f = np.float32
    maps = []
    for c in range(NCORES):
        b, g = c // 2, c % 2
        hs = slice(g * HPC, (g + 1) * HPC)
        maps.append({
            "xqT": _bf16(np.asarray(inputs["inputs_q"][b], f).T),
            "xkT": _bf16(np.asarray(inputs["inputs_k"][b], f).T),
            "xvT": _bf16(np.asarray(inputs["inputs_v"][b], f).T),
            "wq": _bf16(np.asarray(inputs["Wq"], f)[:, hs, :].reshape(D, DH)),
            "wk": _bf16(np.asarray(inputs["Wk"], f)[:, hs, :].reshape(D, DH)),
            "wv": _bf16(np.asarray(inputs["Wv"], f)[:, hs, :].reshape(D, DH)),
            "wo": _bf16(np.asarray(inputs["Wo"], f)[hs].reshape(DH, D)),
            "biases": np.stack(
                [np.asarray(inputs[nm], f)[hs].reshape(DH)[p * P:(p + 1) * P]
                 for nm in ("bq", "bk", "bv") for p in range(PAIRS)], axis=1).copy(),
        })
    return maps


def run_sharded(inputs, **kw):
    """Compile/run on all 8 cores; returns (full_output, BassKernelResults)."""
    from concourse.bass_utils import run_bass_kernel_spmd
    nc = _get_nc()
    res = run_bass_kernel_spmd(nc, _in_maps(inputs), core_ids=list(range(NCORES)), **kw)
    bo = np.asarray(inputs["bo"], np.float32)
    full = np.empty((B, S, D), np.float32)
    for b in range(B):
        full[b] = res.results[2 * b]["out"] + res.results[2 * b + 1]["out"] + bo
    return full, res


def kernel(**inputs) -> np.ndarray:
    full, _ = run_sharded(inputs)
    return full



# revision 7
# speedup vs baseline: 1.6307x; 1.2940x over previous
"""Trainium2 Bass kernel for nn_MultiHeadAttention (B=4, S=2048, D=1024, H=16, HD=64).

Sharding: 8 cores = 4 batches (data parallel) x 2 head-groups of 8 heads
(tensor parallel). Each core computes its batch's QKV projections for its 8
heads, full softmax attention, and the partial output projection for its head
group. The host sums the two head-group partials per batch (the hinted
all-reduce, done at gather time) and adds the output bias.

Per-core kernel layout (all matmuls in float32r at N=512 -> full PE rate):
  - Host pre-transposes activations to X^T [D, S] so projections stream
    contiguously with the contraction dim (features) on partitions.
  - Q/K are produced transposed, Q^T/K^T [dh, tok], one [128, 2048] SBUF tile
    per head pair (head A partitions 0:64, head B 64:128). QKV biases are
    applied for free via the ACT psum->sbuf copy (per-partition bias).
  - Scores are computed transposed S^T[k, q] per head; the two heads of a pair
    run concurrently via PE row packing (tile_position (0,0)/(64,0), K=64).
  - exp runs on ACT straight out of 2-bank PSUM tiles [128, 1024] into an
    E^T SBUF tile [128, 16*512] per head (softmax scale folded into exp).
  - PV: lhsT = V' [128, 65] where column 64 is ones, so the PSUM accumulator
    row 64 collects the softmax denominator Z in the same pass.
  - Normalize: DVE reciprocal of Z + GPSIMD partition_broadcast + DVE multiply,
    writing O^T tiles [dh, tok] which feed the output projection as lhsT.

SBUF pressure is managed with phase-scoped tile pools on the queue allocator:
V'-staging pools die before the Q/K projection pools, which die before the
attention pools (E^T is 64KB/partition), which die before the out-proj pools.
V' and O^T round-trip through DRAM scratch to keep residency low.
"""

import numpy as np
from contextlib import ExitStack

B, S, D = 4, 2048, 1024
H, HD = 16, 64
NCORES = 8
HPC = H // 2            # heads per core = 8
PAIRS = HPC // 2        # head pairs per core = 4
DH = HPC * HD           # per-core head dims = 512
P = 128
TOK_T = S // P          # 16 token tiles of 128
QC = S // 512           # 4 query chunks of 512
KC = S // P             # 16 key chunks of 128
KCG = KC // 2           # 8 exp groups of 2 key chunks
FC = D // P             # 8 feature chunks of 128

_CACHE = {}


def _bf16(x):
    """Round fp32 -> bf16 (RNE) on host so DRAM inputs are bfloat16.
    Halves DMA bytes and, more importantly, halves PE datapath toggling —
    the fp32r baseline spent most of the attention phase HAM-throttled to
    half clock; bf16 operands keep the PE at full rate."""
    import ml_dtypes
    return np.ascontiguousarray(x, np.float32).astype(ml_dtypes.bfloat16)


def _build(reps=1):
    import concourse.bacc as bacc
    import concourse.mybir as mybir
    import concourse.tile as tile

    dt = mybir.dt
    f32 = dt.float32
    bf16 = dt.bfloat16
    AF = mybir.ActivationFunctionType

    nc = bacc.Bacc("TRN2", target_bir_lowering=False, debug=False)

    xqT = nc.dram_tensor("xqT", [D, S], bf16, kind="ExternalInput")
    xkT = nc.dram_tensor("xkT", [D, S], bf16, kind="ExternalInput")
    xvT = nc.dram_tensor("xvT", [D, S], bf16, kind="ExternalInput")
    wq = nc.dram_tensor("wq", [D, DH], bf16, kind="ExternalInput")
    wk = nc.dram_tensor("wk", [D, DH], bf16, kind="ExternalInput")
    wv = nc.dram_tensor("wv", [D, DH], bf16, kind="ExternalInput")
    wo = nc.dram_tensor("wo", [DH, D], bf16, kind="ExternalInput")
    biases = nc.dram_tensor("biases", [P, 3 * PAIRS], f32, kind="ExternalInput")
    out = nc.dram_tensor("out", [S, D], f32, kind="ExternalOutput")

    QCC = 4          # query chunks of 512
    QW = S // QCC    # 512
    KPG = 2          # key tiles per exp group
    NG = KC // KPG   # 8 exp groups per (head, qc)

    def mmr(psum, lhsT, rhs, **kw):
        nc.tensor.matmul(psum, lhsT, rhs, **kw)

    with tile.TileContext(nc, pool_alloc_mode="queue") as tc, ExitStack() as ctx:
        # ---- persistent pools ----
        qt_pool = ctx.enter_context(tc.tile_pool(name="qt", bufs=PAIRS))
        kt_pool = ctx.enter_context(tc.tile_pool(name="kt", bufs=PAIRS))
        vpr_pool = ctx.enter_context(tc.tile_pool(name="vpr", bufs=TOK_T))
        ot_pool = ctx.enter_context(tc.tile_pool(name="ot", bufs=5))
        zr_pool = ctx.enter_context(tc.tile_pool(name="zr", bufs=4))
        zb_pool = ctx.enter_context(tc.tile_pool(name="zb", bufs=4))
        bias_pool = ctx.enter_context(tc.tile_pool(name="bias", bufs=1))
        dram_pool = ctx.enter_context(tc.tile_pool(name="dram", bufs=1, space="DRAM"))
        psp = ctx.enter_context(tc.tile_pool(name="psp", bufs=2, space="PSUM"))
        pssc = ctx.enter_context(tc.tile_pool(name="pssc", bufs=2, space="PSUM"))
        pso = ctx.enter_context(tc.tile_pool(name="pso", bufs=2, space="PSUM"))

        SCALE = 1.0 / float(np.sqrt(HD))
        for rep in range(reps):
          qt_t = [qt_pool.tile([P, S], bf16, name=f"qt_{rep}_{p}", tag="qt")
                  for p in range(PAIRS)]
          kt_t = [kt_pool.tile([P, S], bf16, name=f"kt_{rep}_{p}", tag="kt")
                  for p in range(PAIRS)]
          bias_t = bias_pool.tile([P, 3 * PAIRS], f32, name=f"bias_{rep}", tag="bias")
          nc.sync.dma_start(bias_t[:], biases[:])
          bq_t = {p: bias_t[:, p:p + 1] for p in range(PAIRS)}
          bk_t = {p: bias_t[:, PAIRS + p:PAIRS + p + 1] for p in range(PAIRS)}
          bv_t = {p: bias_t[:, 2 * PAIRS + p:2 * PAIRS + p + 1] for p in range(PAIRS)}

          # ---- attention helpers ----
          def alloc_et(p, qc):
              et = {}
              for hh in range(2):
                  for quarter in range(4):
                      et[(hh, quarter)] = et_pool.tile(
                          [P, 4 * QW], bf16,
                          name=f"et_{rep}_{p}_{qc}_{hh}_{quarter}", tag="et")
              return et

          def scores_exp_group(p, qc, g, et):
              half, goff = g // 2, (g % 2) * KPG * QW  # quarter idx, offset
              psA = pssc.tile([P, KPG * QW], f32,
                              name=f"scA_{rep}_{p}_{qc}_{g}", tag="pssc")
              psB = pssc.tile([P, KPG * QW], f32,
                              name=f"scB_{rep}_{p}_{qc}_{g}", tag="pssc")
              for j in range(KPG):
                  kc = g * KPG + j
                  nc.tensor.matmul(
                      psA[:, j * QW:(j + 1) * QW],
                      kt_t[p][0:64, kc * P:(kc + 1) * P],
                      qt_t[p][0:64, qc * QW:(qc + 1) * QW],
                      start=True, stop=True, tile_position=(0, 0))
                  nc.tensor.matmul(
                      psB[:, j * QW:(j + 1) * QW],
                      kt_t[p][64:128, kc * P:(kc + 1) * P],
                      qt_t[p][64:128, qc * QW:(qc + 1) * QW],
                      start=True, stop=True, tile_position=(64, 0))
              nc.scalar.activation(et[(0, half)][:, goff:goff + KPG * QW],
                                   psA[:], AF.Exp, scale=SCALE)
              nc.scalar.activation(et[(1, half)][:, goff:goff + KPG * QW],
                                   psB[:], AF.Exp, scale=SCALE)

          def scores_exp(p, qc):
              et = alloc_et(p, qc)
              for g in range(NG):
                  scores_exp_group(p, qc, g, et)
              return et

          def pv_chunk(p, qc, et, po_pair, kcs):
              poA, poB = po_pair
              for kc in kcs:
                  half, koff = kc // 4, (kc % 4) * QW
                  cA = (2 * p) * (HD + 1)
                  cB = (2 * p + 1) * (HD + 1)
                  mmr(poA[:], vpr_t[kc][:, cA:cA + HD + 1],
                      et[(0, half)][:, koff:koff + QW],
                      start=(kc == 0), stop=(kc == KC - 1))
                  mmr(poB[:], vpr_t[kc][:, cB:cB + HD + 1],
                      et[(1, half)][:, koff:koff + QW],
                      start=(kc == 0), stop=(kc == KC - 1))

          def normalize(p, qc, po_pair):
              # 1/Z via the fast DVE approx (18 bits, Z ~ O(S) so no edge
              # cases); issue both recips, then both broadcasts, then both
              # muls so the DVE->GPSIMD->DVE chain pipelines across heads.
              # The V bias is folded into the host-side epilogue (softmax
              # rows sum to 1, so + bv commutes past attention and Wo).
              poA, poB = po_pair
              ot_t = ot_pool.tile([P, QW], bf16, name=f"ot_{rep}_{p}_{qc}", tag="ot")
              zr, zb = {}, {}
              for hh, po in ((0, poA), (1, poB)):
                  # approx_fast misreads PSUM operands -> bounce Z through SBUF
                  zc = zr_pool.tile([1, QW], f32,
                                    name=f"zc_{rep}_{p}_{qc}_{hh}", tag="zr")
                  nc.vector.tensor_copy(zc[:], po[64:65, :])
                  zr[hh] = zr_pool.tile([1, QW], f32,
                                        name=f"zr_{rep}_{p}_{qc}_{hh}", tag="zr")
                  nc.vector.reciprocal_approx_fast(zr[hh][:], zc[:])
              for hh in range(2):
                  zb[hh] = zb_pool.tile([64, QW], f32,
                                        name=f"zb_{rep}_{p}_{qc}_{hh}", tag="zb")
                  nc.gpsimd.partition_broadcast(zb[hh][:], zr[hh][:])
              for hh, po in ((0, poA), (1, poB)):
                  nc.vector.tensor_mul(ot_t[hh * 64:(hh + 1) * 64, :],
                                       po[0:64, :], zb[hh][:])
              return ot_t

          def attention(p, qc):
              et = scores_exp(p, qc)
              poA = pso.tile([65, QW], f32, name=f"poA_{rep}_{p}_{qc}", tag="pso")
              poB = pso.tile([65, QW], f32, name=f"poB_{rep}_{p}_{qc}", tag="pso")
              pv_chunk(p, qc, et, (poA, poB), range(KC))
              return normalize(p, qc, (poA, poB))

          vpr_t = []
          # ---- V projection into resident V' tiles (runs first) ----
          with tc.tile_pool(name="wvp", bufs=FC) as wv_pool, \
               tc.tile_pool(name="xsv", bufs=10) as xsv_pool:
              wv_t = []
              for f in range(FC):
                  t = wv_pool.tile([P, 512], bf16, name=f"wv_{rep}_{f}", tag="wv")
                  nc.sync.dma_start(t[:], wv[f * P:(f + 1) * P, :])
                  wv_t.append(t)
              for tcg in range(TOK_T // 4):
                  xv_t = []
                  for f in range(FC):
                      t = xsv_pool.tile([P, 512], bf16,
                                        name=f"xv_{rep}_{tcg}_{f}", tag="xsv")
                      nc.sync.dma_start(t[:], xvT[f * P:(f + 1) * P,
                                                  tcg * 512:(tcg + 1) * 512])
                      xv_t.append(t)
                  for tl in range(4):
                      tci = tcg * 4 + tl
                      ps = psp.tile([P, 512], f32, name=f"psv_{rep}_{tci}", tag="psp")
                      for f in range(FC):
                          mmr(ps[:], xv_t[f][:, tl * P:(tl + 1) * P], wv_t[f][:],
                              start=(f == 0), stop=(f == FC - 1))
                      vt = vpr_pool.tile([P, HPC * (HD + 1)], bf16,
                                         name=f"vpr_{rep}_{tci}", tag="vpr")
                      v3 = vt.rearrange("p (h c) -> p h c", c=HD + 1)
                      nc.gpsimd.memset(v3[:, :, HD:HD + 1], 1.0)
                      nc.vector.tensor_copy(v3[:, :, 0:HD],
                                            ps.rearrange("p (h c) -> p h c", c=HD))
                      vpr_t.append(vt)

          # ---- Q^T / K^T projections (stream X^T once; all pairs) ----
          with tc.tile_pool(name="xs", bufs=10) as xs_pool, \
               tc.tile_pool(name="wqk", bufs=2 * FC) as wqk_pool:
              wq_t, wk_t = {}, {}
              for tc4 in range(QC):
                  for (xT, wT, w_t, dst, b_t, nm) in (
                          (xqT, wq, wq_t, qt_t, bq_t, "q"),
                          (xkT, wk, wk_t, kt_t, bk_t, "k")):
                      x_t = []
                      for f in range(FC):
                          if tc4 == 0:
                              t = wqk_pool.tile([P, 512], bf16,
                                                name=f"w{nm}_{rep}_{f}", tag="wqk")
                              nc.sync.dma_start(t[:], wT[f * P:(f + 1) * P, :])
                              w_t[f] = t
                          t = xs_pool.tile([P, 512], bf16,
                                           name=f"x{nm}_{rep}_{tc4}_{f}", tag="xs")
                          nc.sync.dma_start(t[:], xT[f * P:(f + 1) * P,
                                                     tc4 * 512:(tc4 + 1) * 512])
                          x_t.append(t)
                      for p in range(PAIRS):
                          ps = psp.tile([P, 512], f32,
                                        name=f"ps{nm}_{rep}_{tc4}_{p}", tag="psp")
                          for f in range(FC):
                              mmr(ps[:], w_t[f][:, p * P:(p + 1) * P], x_t[f][:],
                                  start=(f == 0), stop=(f == FC - 1))
                          nc.vector.tensor_scalar_add(
                              dst[p][:, tc4 * 512:(tc4 + 1) * 512], ps[:], b_t[p][:])


          # ---- qc-outer attention sweep; out-proj absorbed per qc unit ----
          with tc.tile_pool(name="et", bufs=8) as et_pool, \
               tc.tile_pool(name="wop", bufs=2 * PAIRS) as wo_pool, \
               tc.tile_pool(name="os", bufs=3) as os_pool:
              wo_t = {}
              for p in range(PAIRS):
                  for dc in range(2):
                      t = wo_pool.tile([P, 512], bf16,
                                       name=f"wo_{rep}_{p}_{dc}", tag="wo")
                      nc.sync.dma_start(t[:], wo[p * P:(p + 1) * P,
                                                 dc * 512:(dc + 1) * 512])
                      wo_t[(p, dc)] = t

              def outproj(qc, ots):
                  for tl in range(QW // P):
                      tci = qc * (QW // P) + tl
                      for dc in range(2):
                          ps = psp.tile([P, 512], f32,
                                        name=f"pout_{rep}_{tci}_{dc}", tag="psp")
                          for pp in range(PAIRS):
                              mmr(ps[:], ots[pp][:, tl * P:(tl + 1) * P],
                                  wo_t[(pp, dc)][:],
                                  start=(pp == 0), stop=(pp == PAIRS - 1))
                          ost = os_pool.tile([P, 512], f32,
                                             name=f"os_{rep}_{tci}_{dc}", tag="os")
                          nc.vector.tensor_copy(ost[:], ps[:])
                          nc.sync.dma_start(out[tci * P:(tci + 1) * P,
                                                dc * 512:(dc + 1) * 512], ost[:])

              for qc in range(QCC):
                  ots = []
                  for p in range(PAIRS):
                      et = scores_exp(p, qc)
                      poA = pso.tile([65, QW], f32,
                                     name=f"poA_{rep}_{p}_{qc}", tag="pso")
                      poB = pso.tile([65, QW], f32,
                                     name=f"poB_{rep}_{p}_{qc}", tag="pso")
                      pv_chunk(p, qc, et, (poA, poB), range(KC))
                      ots.append(normalize(p, qc, (poA, poB)))
                  outproj(qc, ots)
    nc.compile()
    return nc


def _get_nc(reps=1):
    if reps not in _CACHE:
        _CACHE[reps] = _build(reps)
    return _CACHE[reps]


def _in_maps(inputs):
    f = np.float32
    maps = []
    for c in range(NCORES):
        b, g = c // 2, c % 2
        hs = slice(g * HPC, (g + 1) * HPC)
        maps.append({
            "xqT": _bf16(np.asarray(inputs["inputs_q"][b], f).T),
            "xkT": _bf16(np.asarray(inputs["inputs_k"][b], f).T),
            "xvT": _bf16(np.asarray(inputs["inputs_v"][b], f).T),
            "wq": _bf16(np.asarray(inputs["Wq"], f)[:, hs, :].reshape(D, DH)),
            "wk": _bf16(np.asarray(inputs["Wk"], f)[:, hs, :].reshape(D, DH)),
            "wv": _bf16(np.asarray(inputs["Wv"], f)[:, hs, :].reshape(D, DH)),
            "wo": _bf16(np.asarray(inputs["Wo"], f)[hs].reshape(DH, D)),
            "biases": np.stack(
                [np.asarray(inputs[nm], f)[hs].reshape(DH)[p * P:(p + 1) * P]
                 for nm in ("bq", "bk", "bv") for p in range(PAIRS)], axis=1).copy(),
        })
    return maps


def run_sharded(inputs, **kw):
    """Compile/run on all 8 cores; returns (full_output, BassKernelResults)."""
    from concourse.bass_utils import run_bass_kernel_spmd
    nc = _get_nc()
    res = run_bass_kernel_spmd(nc, _in_maps(inputs), core_ids=list(range(NCORES)), **kw)
    # Epilogue: the kernel returns sum_h softmax(s_h) V_h Wo_h per head-group;
    # the V bias rides along as bv @ Wo (softmax rows sum to 1), plus bo.
    bo = np.asarray(inputs["bo"], np.float32)
    bv = np.asarray(inputs["bv"], np.float32)
    wo_f = np.asarray(inputs["Wo"], np.float32)
    bias_full = bo + np.einsum("hd,hdo->o", bv, wo_f)
    full = np.empty((B, S, D), np.float32)
    for b in range(B):
        full[b] = res.results[2 * b]["out"] + res.results[2 * b + 1]["out"] + bias_full
    return full, res


def kernel(**inputs) -> np.ndarray:
    full, _ = run_sharded(inputs)
    return full

